# revision 22
# baseline (speedup 1.0000x reference)
"""Localized embedding layer (separable 5x5 Gaussian stencil) on 8 trn2 cores.

Math: out[i,j,:] = sum_{|di|<=2,|dj|<=2} w(di)w(dj) H[i+di,j+dj,:] / den(i,j)
with w(d) = exp(-c*d^2), c = TILE^2/(2 sigma^2), den(i,j) = r(i)*r(j) rank-1.

Per core (32 output grid rows + 2-row halo each side, zero padded):
  - i-conv (across grid rows)  -> DVE/GPSIMD: 4 fused ops per [128,2*512] row
  - j-conv (across partitions) -> TensorE: one 128x128 banded diag-block
    matmul per output half (1/(r(j)*W_full) folded in); the 4 output columns
    (j=126..129) whose stencil crosses the half boundary are recomputed by a
    batched fix pass: boundary slices of all 32 rows gathered into 2 tiles,
    2 block-diagonal matmuls, written out separately.
  - ScalarE: PSUM->SBUF copy with per-row scale W_full/r(i) (=1 in interior)
  - DMA out (main tiles skip the 4 fixed columns)
"""

import sys
import numpy as np

if "/opt/trn_rl_repo" not in sys.path:
    sys.path.insert(0, "/opt/trn_rl_repo")

G = 256          # grid side
D = 512          # feature dim
P = 2            # grid_step halo
NC = 8           # cores
RPC = G // NC    # rows per core = 32
TILE = 448.0
SIGMA = 200.0

_cache = {}


def _weights():
    c = TILE * TILE / (2.0 * SIGMA * SIGMA)
    return np.exp(-c * np.arange(-P, P + 1) ** 2)   # [w2,w1,1,w1,w2] f64


def _r_vec():
    """r(i) = sum of valid 1D taps at row i (same for columns)."""
    w = _weights()
    r = np.zeros(G)
    for d in range(-P, P + 1):
        lo, hi = max(0, -d), min(G, G - d)
        r[lo:hi] += w[d + P]
    return r


def _host_consts():
    w = _weights()
    r = _r_vec()
    w_full = w.sum()
    # Banded matrix Bp[jout, jin] = w(jout-jin) / (r(jout) * w_full)
    Bp = np.zeros((G, G))
    for d in range(-P, P + 1):
        for jout in range(G):
            jin = jout + d
            if 0 <= jin < G:
                Bp[jout, jin] = w[d + P] / (r[jout] * w_full)
    # main-pass lhsT: diagonal blocks only, layout [k, hm, m]
    wmat = np.zeros((128, 2, 128), dtype=np.float32)
    for hm in range(2):
        blk = Bp[128 * hm:128 * hm + 128, 128 * hm:128 * hm + 128]
        wmat[:, hm, :] = blk.T.astype(np.float32)
    # fix-pass i-conv lhsT [36, 32]: Tstrip[i] = sum_k w[k] * XS[i+k]
    wstrip = np.zeros((RPC + 2 * P, RPC), dtype=np.float32)
    for i in range(RPC):
        for k in range(5):
            wstrip[i + k, i] = w[k]
    # per-core scales
    scales, sfixes = [], []
    for c in range(NC):
        s = (w_full / r[RPC * c: RPC * (c + 1)]).astype(np.float32)
        scales.append(np.broadcast_to(s[None, :], (128, RPC)).copy())
        # strip scale: 1 / (r_i * w_full), per output row (partition)
        sf = np.zeros((128, 1), dtype=np.float32)
        sf[:RPC, 0] = (1.0 / (r[RPC * c: RPC * (c + 1)] * w_full)).astype(np.float32)
        sfixes.append(sf)
    return wmat, wstrip, scales, sfixes


def _build_nc(repeats=1):
    import concourse.bass as bass
    import concourse.mybir as mybir
    import concourse.tile as tile
    from concourse import bacc

    f32 = mybir.dt.float32
    add = mybir.AluOpType.add
    mult = mybir.AluOpType.mult

    w = _weights()
    w1, w2 = float(w[1]), float(w[0])
    NR = RPC + 2 * P

    nc = bacc.Bacc(None, target_bir_lowering=False, debug=False)
    x_dram = nc.declare_dram_parameter("x", [NR, G, D], f32, isOutput=False)
    wm_dram = nc.declare_dram_parameter("wmat", [128, 2, 128], f32, isOutput=False)
    wf_dram = nc.declare_dram_parameter("wstrip", [NR, RPC], f32, isOutput=False)
    sc_dram = nc.declare_dram_parameter("scale", [128, RPC], f32, isOutput=False)
    sf_dram = nc.declare_dram_parameter("sfix", [128, 1], f32, isOutput=False)
    y_dram = nc.declare_dram_parameter("y", [RPC, G, D], f32, isOutput=True)

    # rows whose t1 add goes to gpsimd (load balance: POOL ~38 of 64 adds)
    T1_POOL = {i for i in range(RPC) if i % 8 < 5}

    with tile.TileContext(nc) as tc:
        with (
            tc.tile_pool(name="const", bufs=1) as cpool,
            tc.tile_pool(name="x", bufs=10) as xpool,
            tc.tile_pool(name="tmp", bufs=3) as tpool,
            tc.tile_pool(name="tacc", bufs=4) as tapool,
            tc.tile_pool(name="out", bufs=8) as opool,
            tc.tile_pool(name="fix", bufs=1) as fpool,
            tc.tile_pool(name="psum", bufs=6, space="PSUM") as ppool,
            tc.tile_pool(name="psfix", bufs=2, space="PSUM") as pfpool,
        ):
            wt = cpool.tile([128, 2, 128], f32)
            nc.sync.dma_start(wt[:], wm_dram[:])
            wft = cpool.tile([NR, RPC], f32)
            nc.sync.dma_start(wft[:], wf_dram[:])
            st = cpool.tile([128, RPC], f32)
            nc.sync.dma_start(st[:], sc_dram[:])
            sft = cpool.tile([128, 1], f32)
            nc.sync.dma_start(sft[:], sf_dram[:])

            xt = {}

            def load_row(r):
                t = xpool.tile([128, 2, D], f32, tag="xrow")
                nc.sync.dma_start(
                    t[:], x_dram[r % NR].rearrange("(h p) d -> p h d", p=128)
                )
                xt[r] = t

            for r in range(5):
                load_row(r)

            for rep in range(repeats):
                for i in range(RPC):
                    it = rep * RPC + i
                    if it > 0:
                        load_row(it + 4)
                    a0, a1, a2, a3, a4 = (xt[it + k][:, :, :] for k in range(5))
                    t1 = tpool.tile([128, 2, D], f32, tag="t1")
                    eng1 = nc.gpsimd if i in T1_POOL else nc.vector
                    eng1.tensor_tensor(t1[:], a1, a3, add)
                    t2 = tpool.tile([128, 2, D], f32, tag="t2")
                    nc.gpsimd.tensor_tensor(t2[:], a0, a4, add)
                    t3 = tpool.tile([128, 2, D], f32, tag="t3")
                    nc.vector.scalar_tensor_tensor(t3[:], t2[:], w2 / w1, t1[:], mult, add)
                    tt = tapool.tile([128, 2, D], f32, tag="tacc")
                    nc.vector.scalar_tensor_tensor(tt[:], t3[:], w1, a2, mult, add)
                    for hm in range(2):
                        ps = ppool.tile([128, D], f32, tag="ps")
                        nc.tensor.matmul(
                            ps[:], wt[:, hm, :], tt[:, hm, :], start=True, stop=True
                        )
                        ob = opool.tile([128, D], f32, tag="ob")
                        nc.scalar.mul(ob[:], ps[:], st[:, i:i + 1])
                        if hm == 0:
                            nc.sync.dma_start(y_dram[i, 0:126, :], ob[0:126, :])
                        else:
                            nc.sync.dma_start(y_dram[i, 130:256, :], ob[2:128, :])
                # strip fix pass: recompute jout 126..129 for all 32 rows.
                # XS[r, jj, d] = x[r, 124+jj, d]  (jin strip), partition = row
                xs = fpool.tile([NR, 8, D], f32, tag="xs")
                nc.sync.dma_start(xs[:], x_dram[:, 124:132, :])
                # i-conv on PE: TS[i, jj, d] = sum_k w[k] XS[i+k, jj, d]
                ts = fpool.tile([RPC, 8, D], f32, tag="ts")
                for nchunk in range(8):
                    psf = pfpool.tile([RPC, D], f32, tag="psf")
                    nc.tensor.matmul(
                        psf[:], wft[:], xs[:, nchunk, :], start=True, stop=True
                    )
                    nc.scalar.copy(ts[:, nchunk, :], psf[:])
                # j-conv on free-dim shifts of TS (jout 126..129 <- jj slices)
                f1 = fpool.tile([RPC, 4, D], f32, tag="f1")
                nc.gpsimd.tensor_tensor(f1[:], ts[:, 1:5, :], ts[:, 3:7, :], add)
                f2 = fpool.tile([RPC, 4, D], f32, tag="f2")
                nc.gpsimd.tensor_tensor(f2[:], ts[:, 0:4, :], ts[:, 4:8, :], add)
                f3 = fpool.tile([RPC, 4, D], f32, tag="f3")
                nc.vector.scalar_tensor_tensor(f3[:], f2[:], w2 / w1, f1[:], mult, add)
                f4 = fpool.tile([RPC, 4, D], f32, tag="f4")
                nc.vector.scalar_tensor_tensor(f4[:], f3[:], w1, ts[:, 2:6, :], mult, add)
                fs = fpool.tile([RPC, 4, D], f32, tag="fs")
                nc.scalar.mul(fs[:], f4[:], sft[0:RPC, 0:1])
                nc.sync.dma_start(y_dram[:, 126:130, :], fs[:])
    nc.finalize()
    return nc


def _get_program():
    if "nc" not in _cache:
        _cache["nc"] = _build_nc()
        _cache["consts"] = _host_consts()
    return _cache["nc"], _cache["consts"]


def kernel(H, xy=None):
    from concourse.bass_utils import run_bass_kernel_spmd

    nc, (wmat, wstrip, scales, sfixes) = _get_program()
    H3 = np.ascontiguousarray(H.reshape(G, G, D).astype(np.float32))
    Hp = np.zeros((G + 2 * P, G, D), dtype=np.float32)
    Hp[P:P + G] = H3
    in_maps = []
    for c in range(NC):
        shard = np.ascontiguousarray(Hp[RPC * c: RPC * c + RPC + 2 * P])
        in_maps.append(
            {"x": shard, "wmat": wmat, "wstrip": wstrip,
             "scale": scales[c], "sfix": sfixes[c]}
        )
    res = run_bass_kernel_spmd(nc, in_maps, list(range(NC))).results
    out = np.concatenate([res[c]["y"].reshape(RPC * G, D) for c in range(NC)], axis=0)
    return out


# revision 23
# speedup vs baseline: 1.2351x; 1.2351x over previous
"""Localized embedding layer (separable 5x5 Gaussian stencil) on 8 trn2 cores.

Math: out[i,j,:] = sum_{|di|<=2,|dj|<=2} w(di)w(dj) H[i+di,j+dj,:] / den(i,j)
with w(d) = exp(-c*d^2), c = TILE^2/(2 sigma^2), den(i,j) = r(i)*r(j) rank-1.

Per core (32 output grid rows + 2-row halo each side, zero padded):
  - i-conv (across grid rows)  -> DVE/GPSIMD: 4 fused ops per [128,2*512] row
  - j-conv (across partitions) -> TensorE: one 128x128 banded diag-block
    matmul per output half (1/(r(j)*W_full) folded in); the 4 output columns
    (j=126..129) whose stencil crosses the half boundary are recomputed by a
    batched fix pass: boundary slices of all 32 rows gathered into 2 tiles,
    2 block-diagonal matmuls, written out separately.
  - ScalarE: PSUM->SBUF copy with per-row scale W_full/r(i) (=1 in interior)
  - DMA out (main tiles skip the 4 fixed columns)
"""

import sys
import numpy as np

if "/opt/trn_rl_repo" not in sys.path:
    sys.path.insert(0, "/opt/trn_rl_repo")

G = 256          # grid side
D = 512          # feature dim
P = 2            # grid_step halo
NC = 8           # cores
RPC = G // NC    # rows per core = 32
TILE = 448.0
SIGMA = 200.0

_cache = {}


def _weights():
    c = TILE * TILE / (2.0 * SIGMA * SIGMA)
    return np.exp(-c * np.arange(-P, P + 1) ** 2)   # [w2,w1,1,w1,w2] f64


def _r_vec():
    """r(i) = sum of valid 1D taps at row i (same for columns)."""
    w = _weights()
    r = np.zeros(G)
    for d in range(-P, P + 1):
        lo, hi = max(0, -d), min(G, G - d)
        r[lo:hi] += w[d + P]
    return r


def _host_consts():
    w = _weights()
    r = _r_vec()
    w_full = w.sum()
    # Banded matrix Bp[jout, jin] = w(jout-jin) / (r(jout) * w_full)
    Bp = np.zeros((G, G))
    for d in range(-P, P + 1):
        for jout in range(G):
            jin = jout + d
            if 0 <= jin < G:
                Bp[jout, jin] = w[d + P] / (r[jout] * w_full)
    # main-pass lhsT: diagonal blocks only, layout [k, hm, m]
    wmat = np.zeros((128, 2, 128), dtype=np.float32)
    for hm in range(2):
        blk = Bp[128 * hm:128 * hm + 128, 128 * hm:128 * hm + 128]
        wmat[:, hm, :] = blk.T.astype(np.float32)
    # fix-pass i-conv lhsT [36, 32]: Tstrip[i] = sum_k w[k] * XS[i+k]
    wstrip = np.zeros((RPC + 2 * P, RPC), dtype=np.float32)
    for i in range(RPC):
        for k in range(5):
            wstrip[i + k, i] = w[k]
    # per-core scales
    scales, sfixes = [], []
    for c in range(NC):
        s = (w_full / r[RPC * c: RPC * (c + 1)]).astype(np.float32)
        scales.append(np.broadcast_to(s[None, :], (128, RPC)).copy())
        # strip scale: 1 / (r_i * w_full), per output row (partition)
        sf = np.zeros((128, 1), dtype=np.float32)
        sf[:RPC, 0] = (1.0 / (r[RPC * c: RPC * (c + 1)] * w_full)).astype(np.float32)
        sfixes.append(sf)
    return wmat, wstrip, scales, sfixes


def _build_nc(repeats=1):
    import concourse.bass as bass
    import concourse.mybir as mybir
    import concourse.tile as tile
    from concourse import bacc

    f32 = mybir.dt.float32
    add = mybir.AluOpType.add
    mult = mybir.AluOpType.mult

    w = _weights()
    w1, w2 = float(w[1]), float(w[0])
    NR = RPC + 2 * P

    nc = bacc.Bacc(None, target_bir_lowering=False, debug=False)
    x_dram = nc.declare_dram_parameter("x", [NR, G, D], f32, isOutput=False)
    wm_dram = nc.declare_dram_parameter("wmat", [128, 2, 128], f32, isOutput=False)
    wf_dram = nc.declare_dram_parameter("wstrip", [NR, RPC], f32, isOutput=False)
    sc_dram = nc.declare_dram_parameter("scale", [128, RPC], f32, isOutput=False)
    sf_dram = nc.declare_dram_parameter("sfix", [128, 1], f32, isOutput=False)
    y_dram = nc.declare_dram_parameter("y", [RPC, G, D], f32, isOutput=True)

    # rows whose t1 add goes to gpsimd (load balance: POOL ~38 of 64 adds)
    T1_POOL = {i for i in range(RPC) if i % 5 == 0}

    with tile.TileContext(nc) as tc:
        with (
            tc.tile_pool(name="const", bufs=1) as cpool,
            tc.tile_pool(name="x", bufs=10) as xpool,
            tc.tile_pool(name="tmp", bufs=3) as tpool,
            tc.tile_pool(name="tacc", bufs=4) as tapool,
            tc.tile_pool(name="out", bufs=8) as opool,
            tc.tile_pool(name="fix", bufs=1) as fpool,
            tc.tile_pool(name="psum", bufs=6, space="PSUM") as ppool,
            tc.tile_pool(name="psfix", bufs=2, space="PSUM") as pfpool,
        ):
            wt = cpool.tile([128, 2, 128], f32)
            nc.sync.dma_start(wt[:], wm_dram[:])
            wft = cpool.tile([NR, RPC], f32)
            nc.sync.dma_start(wft[:], wf_dram[:])
            st = cpool.tile([128, RPC], f32)
            nc.sync.dma_start(st[:], sc_dram[:])
            sft = cpool.tile([128, 1], f32)
            nc.sync.dma_start(sft[:], sf_dram[:])

            xt = {}

            def load_row(r):
                t = xpool.tile([128, 2, D], f32, tag="xrow")
                nc.sync.dma_start(
                    t[:], x_dram[r % NR].rearrange("(h p) d -> p h d", p=128)
                )
                xt[r] = t

            for r in range(5):
                load_row(r)

            for rep in range(repeats):
                for i in range(RPC):
                    it = rep * RPC + i
                    if it > 0:
                        load_row(it + 4)
                    a0, a1, a2, a3, a4 = (xt[it + k][:, :, :] for k in range(5))
                    t1 = tpool.tile([128, 2, D], f32, tag="t1")
                    eng1 = nc.gpsimd if i in T1_POOL else nc.vector
                    eng1.tensor_tensor(t1[:], a1, a3, add)
                    t2 = tpool.tile([128, 2, D], f32, tag="t2")
                    nc.gpsimd.tensor_tensor(t2[:], a0, a4, add)
                    t3 = tpool.tile([128, 2, D], f32, tag="t3")
                    nc.vector.scalar_tensor_tensor(t3[:], t2[:], w2 / w1, t1[:], mult, add)
                    tt = tapool.tile([128, 2, D], f32, tag="tacc")
                    nc.vector.scalar_tensor_tensor(tt[:], t3[:], w1, a2, mult, add)
                    for hm in range(2):
                        ps = ppool.tile([128, D], f32, tag="ps")
                        nc.tensor.matmul(
                            ps[:], wt[:, hm, :], tt[:, hm, :], start=True, stop=True
                        )
                        ob = opool.tile([128, D], f32, tag="ob")
                        nc.scalar.mul(ob[:], ps[:], st[:, i:i + 1])
                        if hm == 0:
                            nc.sync.dma_start(y_dram[i, 0:126, :], ob[0:126, :])
                        else:
                            nc.sync.dma_start(y_dram[i, 130:256, :], ob[2:128, :])
                # strip fix pass: recompute jout 126..129 for all 32 rows.
                # XS[r, jj, d] = x[r, 124+jj, d]  (jin strip), partition = row
                xs = fpool.tile([NR, 8, D], f32, tag="xs")
                nc.sync.dma_start(xs[:], x_dram[:, 124:132, :])
                # i-conv on PE: TS[i, jj, d] = sum_k w[k] XS[i+k, jj, d]
                ts = fpool.tile([RPC, 8, D], f32, tag="ts")
                for nchunk in range(8):
                    psf = pfpool.tile([RPC, D], f32, tag="psf")
                    nc.tensor.matmul(
                        psf[:], wft[:], xs[:, nchunk, :], start=True, stop=True
                    )
                    nc.scalar.copy(ts[:, nchunk, :], psf[:])
                # j-conv on free-dim shifts of TS (jout 126..129 <- jj slices)
                f1 = fpool.tile([RPC, 4, D], f32, tag="f1")
                nc.gpsimd.tensor_tensor(f1[:], ts[:, 1:5, :], ts[:, 3:7, :], add)
                f2 = fpool.tile([RPC, 4, D], f32, tag="f2")
                nc.gpsimd.tensor_tensor(f2[:], ts[:, 0:4, :], ts[:, 4:8, :], add)
                f3 = fpool.tile([RPC, 4, D], f32, tag="f3")
                nc.vector.scalar_tensor_tensor(f3[:], f2[:], w2 / w1, f1[:], mult, add)
                f4 = fpool.tile([RPC, 4, D], f32, tag="f4")
                nc.vector.scalar_tensor_tensor(f4[:], f3[:], w1, ts[:, 2:6, :], mult, add)
                fs = fpool.tile([RPC, 4, D], f32, tag="fs")
                nc.scalar.mul(fs[:], f4[:], sft[0:RPC, 0:1])
                nc.sync.dma_start(y_dram[:, 126:130, :], fs[:])
    nc.finalize()
    return nc


def _get_program():
    if "nc" not in _cache:
        _cache["nc"] = _build_nc()
        _cache["consts"] = _host_consts()
    return _cache["nc"], _cache["consts"]


def kernel(H, xy=None):
    from concourse.bass_utils import run_bass_kernel_spmd

    nc, (wmat, wstrip, scales, sfixes) = _get_program()
    H3 = np.ascontiguousarray(H.reshape(G, G, D).astype(np.float32))
    Hp = np.zeros((G + 2 * P, G, D), dtype=np.float32)
    Hp[P:P + G] = H3
    in_maps = []
    for c in range(NC):
        shard = np.ascontiguousarray(Hp[RPC * c: RPC * c + RPC + 2 * P])
        in_maps.append(
            {"x": shard, "wmat": wmat, "wstrip": wstrip,
             "scale": scales[c], "sfix": sfixes[c]}
        )
    res = run_bass_kernel_spmd(nc, in_maps, list(range(NC))).results
    out = np.concatenate([res[c]["y"].reshape(RPC * G, D) for c in range(NC)], axis=0)
    return out
